# revision 29
# baseline (speedup 1.0000x reference)
"""CrossAttention Trainium2 kernel (v2 — fp16 datapath, pipelined phases).

Full inputs in, full output out. Data-parallel over batch: core b computes
batch item b of 8.

Per-core math (all layouts transposed so the PE contraction dim is always
the partition dim, with no on-chip transposes):
  QT[d, q] = (Wq*scale @ q_b^T)      lhsT=WqT chunks, rhs=q_b^T   (fp16)
  KT[d, k] = (Wk @ kv_b^T)
  V[k, d]  = (kv_b @ Wv^T)           fp16, with ones column for rowsum
  S^T[k, q] = K Q^T                  per head (64-dim contraction)
  P^T = exp(S^T - ln256) * exp_posT  (exp_pos precomputed on host, fp16)
  O^T[d, q] (+rowsum row 64) = V_aug^T-contraction over k, psum-accumulated
  X^T = O^T[0:64] * (1/rowsum) broadcast
  out[q, e] = X^T.T @ WprojT + bias

Pipelining: emission order is Qproj, Kproj, S(0), Vproj, then per head h:
S(h) followed by O(h-1), so the PE has O-matmul work while the scalar
engine runs exp over S(h).  One PSUM pool spans all phases (tags sp/op,
2+2 slots = 8 banks).  Rowsum reciprocals run on a [128, 16] DMA-gathered
layout (DVE reciprocal is 8 cyc/elem *per lane*).
"""

import numpy as np

B, L, DIM, H, HD = 8, 1024, 768, 12, 64
NCORES = 8
CP = DIM // 128  # 6 chunks of the contraction/feature dim
KC = L // 128    # 8 k-chunks
SCALE = HD ** -0.5
LN_OFF = float(np.log(256.0))

_CACHE = {}


def _build():
    import concourse.bass as bass
    import concourse.mybir as mybir
    import concourse.tile as tile
    from concourse import bacc

    f32 = mybir.dt.float32
    f16 = mybir.dt.float16
    AF = mybir.ActivationFunctionType

    nc = bacc.Bacc("TRN2", target_bir_lowering=False, debug=False)

    qT = nc.dram_tensor("qT", [DIM, L], f16, kind="ExternalInput")
    kvT = nc.dram_tensor("kvT", [DIM, L], f16, kind="ExternalInput")
    wq = nc.dram_tensor("wq", [DIM, DIM], f16, kind="ExternalInput")  # [c, d]
    wk = nc.dram_tensor("wk", [DIM, DIM], f16, kind="ExternalInput")  # [c, d]
    wv = nc.dram_tensor("wv", [DIM, DIM], f16, kind="ExternalInput")  # [c, d]
    wp = nc.dram_tensor("wp", [DIM, DIM], f16, kind="ExternalInput")  # [d, e]
    bias = nc.dram_tensor("bias", [128, DIM], f32, kind="ExternalInput")
    epos = nc.dram_tensor("epos", [H, L, L], f16, kind="ExternalInput")  # [h,k,q]
    out = nc.dram_tensor("out", [L, DIM], f32, kind="ExternalOutput")
    rscr = nc.dram_tensor("rs_scratch", [H, L], f32)       # rowsums
    rscr2 = nc.dram_tensor("rs_recip", [H, 128, 8], f32)   # reciprocals, q=p*8+a
    pscr = nc.dram_tensor("partial_scratch", [KC, 128, DIM], f32)  # out-proj d0-2

    with tile.TileContext(nc) as tc:
        with (
            tc.tile_pool(name="persist", bufs=1) as persist,
            tc.tile_pool(name="ps", bufs=2, space="PSUM") as PS,
        ):
            QT = persist.tile([128, CP, L], f16)   # pair p: heads 2p, 2p+1
            KT = persist.tile([128, CP, L], f16)
            XT = persist.tile([128, CP, L], f16)
            Vt = [
                persist.tile([128, H, HD + 1], f16, name=f"Vt{k}", tag=f"V{k}")
                for k in range(KC)
            ]
            wp_sb = persist.tile([128, CP, DIM], f16)
            bias_bc = persist.tile([128, DIM], f32)
            expb = persist.tile([128, 1], f32)
            nc.vector.memset(expb[:], -LN_OFF)

            nc.sync.dma_start(wp_sb[:], wp.rearrange("(a p) d -> p a d", p=128))
            nc.sync.dma_start(bias_bc[:], bias[:])

            with (
                tc.tile_pool(name="ph1", bufs=1) as ph1,
                tc.tile_pool(name="eposp", bufs=6) as eposp,
                tc.tile_pool(name="prp", bufs=3) as prp,
                tc.tile_pool(name="ptp", bufs=18) as ptp,
                tc.tile_pool(name="xtup", bufs=5) as xtup,
                tc.tile_pool(name="bcp", bufs=2) as bcp,
                tc.tile_pool(name="rsp", bufs=2) as rsp,
                tc.tile_pool(name="outp", bufs=3) as outp,
            ):
                q_sb = ph1.tile([128, CP, L], f16)
                kv_sb = ph1.tile([128, CP, L], f16)
                wq_sb = ph1.tile([128, CP, DIM], f16)
                wk_sb = ph1.tile([128, CP, DIM], f16)
                wv_sb = ph1.tile([128, CP, DIM], f16)

                # per-chunk DMAs so the first projection matmul starts after
                # ~2 chunk loads instead of the full 1.5 MB transfer
                wq_v = wq.rearrange("(a p) d -> p a d", p=128)
                q_v = qT.rearrange("(a p) q -> p a q", p=128)
                wk_v = wk.rearrange("(a p) d -> p a d", p=128)
                kv_v = kvT.rearrange("(a p) q -> p a q", p=128)
                for c in range(CP):
                    nc.sync.dma_start(wq_sb[:, c, :], wq_v[:, c, :])
                    nc.sync.dma_start(q_sb[:, c, :], q_v[:, c, :])
                for c in range(CP):
                    nc.sync.dma_start(wk_sb[:, c, :], wk_v[:, c, :])
                    nc.sync.dma_start(kv_sb[:, c, :], kv_v[:, c, :])
                nc.sync.dma_start(wv_sb[:], wv.rearrange("(a p) d -> p a d", p=128))

                def emit_QK(p):
                    """Q and K projection for head-pair p."""
                    for w_sb, x_sb, dst in ((wq_sb, q_sb, QT), (wk_sb, kv_sb, KT)):
                        ps = PS.tile([128, L], f32, tag="sp")
                        for c in range(CP):
                            for hf in range(2):
                                nc.tensor.matmul(
                                    ps[:, hf * 512:(hf + 1) * 512],
                                    w_sb[:, c, p * 128:(p + 1) * 128],
                                    x_sb[:, c, hf * 512:(hf + 1) * 512],
                                    start=(c == 0),
                                    stop=(c == CP - 1),
                                )
                        nc.vector.tensor_copy(dst[:, p, :], ps[:])

                # ---------------- attention + V-proj, software-pipelined ----
                pt_tiles = {}    # (h, k) -> pt tile
                xtu = [None] * H
                rs_pair = {}     # pair p -> [128, 2, 8] rowsum tile

                def emit_S_chunk(h, k):
                    """One S^T k-chunk for head h: matmul, exp, mul by epos."""
                    p, sub = divmod(h, 2)
                    s_ps = PS.tile([128, L], f32, tag="sp")
                    for hf in range(2):
                        nc.tensor.matmul(
                            s_ps[:, hf * 512:(hf + 1) * 512],
                            KT[sub * 64:(sub + 1) * 64, p, k * 128:(k + 1) * 128],
                            QT[sub * 64:(sub + 1) * 64, p, hf * 512:(hf + 1) * 512],
                        )
                    pr = prp.tile([128, L], f16, tag="pr")
                    nc.scalar.activation(pr[:], s_ps[:], AF.Exp, bias=expb[:])
                    ep = eposp.tile([128, L], f16, tag="ep")
                    nc.sync.dma_start(ep[:], epos[h, k * 128:(k + 1) * 128, :])
                    pt = ptp.tile([128, L], f16, name=f"pt{h}_{k}", tag="pt")
                    # offload one multiply per head to the otherwise-idle GpSimd
                    eng = nc.gpsimd if k == 3 else nc.vector
                    eng.tensor_mul(pt[:], pr[:], ep[:])
                    pt_tiles[(h, k)] = pt

                def emit_V(k):
                    ps = PS.tile([128, DIM], f32, tag="op")
                    for c in range(CP):
                        for lo, sz in ((0, 512), (512, 256)):
                            nc.tensor.matmul(
                                ps[:, lo:lo + sz],
                                kv_sb[:, c, k * 128:(k + 1) * 128],
                                wv_sb[:, c, lo:lo + sz],
                                start=(c == 0),
                                stop=(c == CP - 1),
                            )
                    nc.vector.memset(Vt[k][:, :, HD:HD + 1], 1.0)
                    nc.vector.tensor_copy(
                        Vt[k][:, :, 0:HD],
                        ps[:].rearrange("p (h d) -> p h d", d=HD),
                    )

                o_ps_cur = [None]

                def emit_O_chunk(h, k):
                    """One O^T accumulation step (k-chunk) for head h."""
                    if k == 0:
                        o_ps_cur[0] = PS.tile([65, L], f32, name=f"ops{h}", tag="op")
                    o_ps = o_ps_cur[0]
                    pt = pt_tiles.pop((h, k))
                    for hf in range(2):
                        nc.tensor.matmul(
                            o_ps[:, hf * 512:(hf + 1) * 512],
                            Vt[k][:, h, :],
                            pt[:, hf * 512:(hf + 1) * 512],
                            start=(k == 0),
                            stop=(k == KC - 1),
                        )
                    if k == KC - 1:
                        xtu[h] = xtup.tile([65, L], f32, name=f"xtu{h}", tag="xtu")
                        nc.vector.tensor_copy(xtu[h][:], o_ps[:])
                        nc.sync.dma_start(rscr[h:h + 1, :], xtu[h][64:65, :])

                # ---- normalize machinery, staged to avoid queue-head stalls:
                # stage A (iter 2p+2): gather pair rowsums to [128, 2, 8]
                # stage B (iter 2p+2, late): reciprocal + scatter + bc loads
                # stage C (iter 2p+3): normalize muls (GpSimd; bc landed)
                bcs = {}

                def norm_stage_A(p):
                    h0 = 2 * p
                    rs_pair[p] = rsp.tile([128, 2, 8], f32, name=f"rs{p}", tag="rs")
                    nc.sync.dma_start(
                        rs_pair[p][:],
                        rscr[h0:h0 + 2, :].rearrange("h (p a) -> p h a", p=128),
                    )

                def norm_stage_B(p, recip_eng):
                    rc_t = rsp.tile([128, 2, 8], f32, name=f"rc{p}", tag="rc")
                    recip_eng.reciprocal(rc_t[:], rs_pair.pop(p)[:])
                    for sub in range(2):
                        h = 2 * p + sub
                        nc.sync.dma_start(rscr2[h], rc_t[:, sub, :])
                        bc = bcp.tile([64, L], f32, name=f"bc{h}", tag="bc")
                        nc.sync.dma_start(
                            bc[:],
                            rscr2[h]
                            .rearrange("p a -> (p a)")
                            .unsqueeze(0)
                            .broadcast_to([64, L]),
                        )
                        bcs[h] = bc

                def norm_stage_C(p, eng):
                    for sub in range(2):
                        h = 2 * p + sub
                        eng.tensor_mul(
                            XT[sub * 64:(sub + 1) * 64, p, :],
                            xtu[h][0:64, :],
                            bcs.pop(h),
                        )
                        xtu[h] = None

                # ---- head-pair projections spread across iterations as PE
                # filler on the spare "op" PSUM slot: 2 MMs per k-step.
                def make_proj_job(p, which):
                    return {"p": p, "which": which, "ps": None}

                def emit_proj_step(job, c):
                    if job is None or c >= CP:
                        return
                    p = job["p"]
                    w_sb, x_sb, dst = (
                        (wq_sb, q_sb, QT)
                        if job["which"] == "Q"
                        else (wk_sb, kv_sb, KT)
                    )
                    if c == 0:
                        job["ps"] = PS.tile(
                            [128, L], f32, name=f"pj{p}{job['which']}", tag="op"
                        )
                    for hf in range(2):
                        nc.tensor.matmul(
                            job["ps"][:, hf * 512:(hf + 1) * 512],
                            w_sb[:, c, p * 128:(p + 1) * 128],
                            x_sb[:, c, hf * 512:(hf + 1) * 512],
                            start=(c == 0),
                            stop=(c == CP - 1),
                        )
                    if c == CP - 1:
                        nc.vector.tensor_copy(dst[:, p, :], job["ps"][:])

                # ---------------- front ----------------
                emit_QK(0)
                for k0 in range(0, KC, 2):
                    emit_S_chunk(0, k0)
                    emit_S_chunk(0, k0 + 1)
                    emit_V(k0)
                    emit_V(k0 + 1)
                # pair-1 projections spread through the S(1) stream so ACT
                # stays continuously fed
                jq1, jk1 = make_proj_job(1, "Q"), make_proj_job(1, "K")
                for k in range(KC):
                    emit_S_chunk(1, k)
                    emit_proj_step(jq1, k)
                for c in range(CP):
                    emit_proj_step(jk1, c)

                # Q/K projections of pair p: Q at iter 2p-3, K at 2p-2
                proj_sched = {}
                for p in range(2, CP):
                    proj_sched[2 * p - 3] = make_proj_job(p, "Q")
                    proj_sched[2 * p - 2] = make_proj_job(p, "K")

                # ---------------- steady state ----------------
                def emit_outproj_half1(qc):
                    """out-proj d-chunks 0-2 for one q-chunk; partial (+bias)
                    bounced to DRAM. Runs mid-attention on the spare op slot."""
                    ps = PS.tile([128, DIM], f32, name=f"po{qc}", tag="op")
                    for d in range(3):
                        for lo, sz in ((0, 512), (512, 256)):
                            nc.tensor.matmul(
                                ps[:, lo:lo + sz],
                                XT[:, d, qc * 128:(qc + 1) * 128],
                                wp_sb[:, d, lo:lo + sz],
                                start=(d == 0),
                                stop=(d == 2),
                            )
                    tmp = outp.tile([128, DIM], f32, name=f"tm{qc}", tag="ot")
                    nc.vector.tensor_add(tmp[:], ps[:], bias_bc[:])
                    nc.sync.dma_start(pscr[qc], tmp[:])

                outproj_q = iter(range(KC))
                for h in range(1, H):
                    job = proj_sched.get(h)
                    for k0 in range(0, KC, 2):
                        # batch same-PE-array-mode matmuls (S = 64-row tiled,
                        # O/proj = full 128) to halve mode-switch drains
                        for k in (k0, k0 + 1):
                            if h + 1 < H:
                                emit_S_chunk(h + 1, k)
                        for k in (k0, k0 + 1):
                            emit_O_chunk(h - 1, k)
                            emit_proj_step(job, k)
                        k = k0 + 1
                        # pair p: rowsums complete at end of iter 2p+2 ->
                        # A/B at iter 2p+3 (odd), C at iter 2p+4 (even)
                        if k == 1 and h % 2 == 1 and h >= 3:
                            norm_stage_A((h - 3) // 2)
                        if k == 5 and h % 2 == 1 and h >= 3:
                            norm_stage_B((h - 3) // 2, nc.vector)
                        if k == 5 and h % 2 == 0 and h >= 4:
                            norm_stage_C((h - 4) // 2, nc.gpsimd)
                        # out-proj first halves as PE filler in iters 9-11
                        # (pairs 0-2 normalized by iter 8; no proj jobs left)
                        if h >= 9 and k in (1, 3, 7):
                            qc = next(outproj_q, None)
                            if qc is not None:
                                emit_outproj_half1(qc)
                for k in range(KC):
                    emit_O_chunk(H - 1, k)
                # tail: finish pairs 4 and 5 immediately on DVE
                norm_stage_C(H // 2 - 2, nc.vector)
                norm_stage_A(H // 2 - 1)
                norm_stage_B(H // 2 - 1, nc.vector)
                norm_stage_C(H // 2 - 1, nc.vector)

                # ---------------- phase 3: output projection (d 3-5) --------
                # alternate PSUM tags so 4 accumulations are in flight
                for qc in range(KC):
                    tmp = outp.tile([128, DIM], f32, name=f"tn{qc}", tag="ot")
                    nc.sync.dma_start(tmp[:], pscr[qc])
                    ps = PS.tile([128, DIM], f32, tag="op" if qc % 2 == 0 else "sp")
                    for d in range(3, CP):
                        for lo, sz in ((0, 512), (512, 256)):
                            nc.tensor.matmul(
                                ps[:, lo:lo + sz],
                                XT[:, d, qc * 128:(qc + 1) * 128],
                                wp_sb[:, d, lo:lo + sz],
                                start=(d == 3),
                                stop=(d == CP - 1),
                            )
                    ot = outp.tile([128, DIM], f32, name=f"oo{qc}", tag="ot")
                    nc.vector.tensor_add(ot[:], ps[:], tmp[:])
                    nc.sync.dma_start(out[qc * 128:(qc + 1) * 128, :], ot[:])

    nc.compile()
    return nc


def _get_nc():
    if "nc" not in _CACHE:
        _CACHE["nc"] = _build()
    return _CACHE["nc"]


def _host_prep(q, kv, attn_pos, Wq, Wkv, Wproj, bproj):
    q = np.asarray(q, dtype=np.float32)
    kv = np.asarray(kv, dtype=np.float32)
    attn_pos = np.asarray(attn_pos, dtype=np.float32)
    Wq = np.asarray(Wq, dtype=np.float32)
    Wkv = np.asarray(Wkv, dtype=np.float32)
    Wproj = np.asarray(Wproj, dtype=np.float32)
    bproj = np.asarray(bproj, dtype=np.float32)

    wq = np.ascontiguousarray((Wq * SCALE).T).astype(np.float16)   # [c, d]
    wk = np.ascontiguousarray(Wkv[:DIM].T).astype(np.float16)      # [c, d]
    wv = np.ascontiguousarray(Wkv[DIM:].T).astype(np.float16)      # [c, d]
    wp = np.ascontiguousarray(Wproj.T).astype(np.float16)          # [d, e]
    bias = np.ascontiguousarray(np.tile(bproj[None, :], (128, 1)))
    # epos[h, k, q] = exp(attn_pos[0, h, q, k])
    epos = np.ascontiguousarray(
        np.exp(attn_pos[0]).transpose(0, 2, 1)
    ).astype(np.float16)

    qT = np.ascontiguousarray(q.transpose(0, 2, 1)).astype(np.float16)   # [B, c, L]
    kvT = np.ascontiguousarray(kv.transpose(0, 2, 1)).astype(np.float16)

    shared = {"wq": wq, "wk": wk, "wv": wv, "wp": wp, "bias": bias, "epos": epos}
    in_maps = []
    for b in range(B):
        m = dict(shared)
        m["qT"] = qT[b]
        m["kvT"] = kvT[b]
        in_maps.append(m)
    return in_maps


def kernel(q, kv, attn_pos, Wq, Wkv, Wproj, bproj):
    from concourse.bass_utils import run_bass_kernel_spmd

    nc = _get_nc()
    in_maps = _host_prep(q, kv, attn_pos, Wq, Wkv, Wproj, bproj)
    res = run_bass_kernel_spmd(nc, in_maps, list(range(NCORES)))
    return np.stack([res.results[b]["out"] for b in range(B)], axis=0)


# revision 34
# speedup vs baseline: 1.1462x; 1.1462x over previous
"""CrossAttention Trainium2 kernel (v2 — fp16 datapath, pipelined phases).

Full inputs in, full output out. Data-parallel over batch: core b computes
batch item b of 8.

Per-core math (all layouts transposed so the PE contraction dim is always
the partition dim, with no on-chip transposes):
  QT[d, q] = (Wq*scale @ q_b^T)      lhsT=WqT chunks, rhs=q_b^T   (fp16)
  KT[d, k] = (Wk @ kv_b^T)
  V[k, d]  = (kv_b @ Wv^T)           fp16, with ones column for rowsum
  S^T[k, q] = K Q^T                  per head (64-dim contraction)
  P^T = exp(S^T - ln256) * exp_posT  (exp_pos precomputed on host, fp16)
  O^T[d, q] (+rowsum row 64) = V_aug^T-contraction over k, psum-accumulated
  X^T = O^T[0:64] * (1/rowsum) broadcast
  out[q, e] = X^T.T @ WprojT + bias

Pipelining: emission order is Qproj, Kproj, S(0), Vproj, then per head h:
S(h) followed by O(h-1), so the PE has O-matmul work while the scalar
engine runs exp over S(h).  One PSUM pool spans all phases (tags sp/op,
2+2 slots = 8 banks).  Rowsum reciprocals run on a [128, 16] DMA-gathered
layout (DVE reciprocal is 8 cyc/elem *per lane*).
"""

import numpy as np

B, L, DIM, H, HD = 8, 1024, 768, 12, 64
NCORES = 8
CP = DIM // 128  # 6 chunks of the contraction/feature dim
KC = L // 128    # 8 k-chunks
SCALE = HD ** -0.5
LN_OFF = float(np.log(256.0))

_CACHE = {}


def _build():
    import concourse.bass as bass
    import concourse.mybir as mybir
    import concourse.tile as tile
    from concourse import bacc

    f32 = mybir.dt.float32
    f16 = mybir.dt.float16
    AF = mybir.ActivationFunctionType

    nc = bacc.Bacc("TRN2", target_bir_lowering=False, debug=False)

    qT = nc.dram_tensor("qT", [DIM, L], f16, kind="ExternalInput")
    kvT = nc.dram_tensor("kvT", [DIM, L], f16, kind="ExternalInput")
    wq = nc.dram_tensor("wq", [DIM, DIM], f16, kind="ExternalInput")  # [c, d]
    wk = nc.dram_tensor("wk", [DIM, DIM], f16, kind="ExternalInput")  # [c, d]
    wv = nc.dram_tensor("wv", [DIM, DIM], f16, kind="ExternalInput")  # [c, d]
    wp = nc.dram_tensor("wp", [DIM, DIM], f16, kind="ExternalInput")  # [d, e]
    bias = nc.dram_tensor("bias", [128, DIM], f32, kind="ExternalInput")
    epos = nc.dram_tensor("epos", [H, L, L], f16, kind="ExternalInput")  # [h,k,q]
    out = nc.dram_tensor("out", [L, DIM], f32, kind="ExternalOutput")
    rscr = nc.dram_tensor("rs_scratch", [H, L], f32)       # rowsums
    rscr2 = nc.dram_tensor("rs_recip", [H, 128, 8], f32)   # reciprocals, q=p*8+a

    with tile.TileContext(nc) as tc:
        with (
            tc.tile_pool(name="persist", bufs=1) as persist,
            tc.tile_pool(name="ps", bufs=2, space="PSUM") as PS,
        ):
            QT = persist.tile([128, CP, L], f16)   # pair p: heads 2p, 2p+1
            KT = persist.tile([128, CP, L], f16)
            XT = persist.tile([128, CP, L], f16)
            Vt = [
                persist.tile([128, H, HD + 1], f16, name=f"Vt{k}", tag=f"V{k}")
                for k in range(KC)
            ]
            wp_sb = persist.tile([128, CP, DIM], f16)
            bias_bc = persist.tile([128, DIM], f32)
            expb = persist.tile([128, 1], f32)
            nc.vector.memset(expb[:], -LN_OFF)

            nc.sync.dma_start(wp_sb[:], wp.rearrange("(a p) d -> p a d", p=128))
            nc.sync.dma_start(bias_bc[:], bias[:])

            with (
                tc.tile_pool(name="ph1", bufs=1) as ph1,
                tc.tile_pool(name="eposp", bufs=8) as eposp,
                tc.tile_pool(name="prp", bufs=3) as prp,
                tc.tile_pool(name="ptp", bufs=18) as ptp,
                tc.tile_pool(name="xtup", bufs=5) as xtup,
                tc.tile_pool(name="bcp", bufs=3) as bcp,
                tc.tile_pool(name="rsp", bufs=2) as rsp,
                tc.tile_pool(name="outp", bufs=2) as outp,
            ):
                q_sb = ph1.tile([128, CP, L], f16)
                kv_sb = ph1.tile([128, CP, L], f16)
                wq_sb = ph1.tile([128, CP, DIM], f16)
                wk_sb = ph1.tile([128, CP, DIM], f16)
                wv_sb = ph1.tile([128, CP, DIM], f16)

                # per-chunk DMAs so the first projection matmul starts after
                # ~2 chunk loads instead of the full 1.5 MB transfer
                wq_v = wq.rearrange("(a p) d -> p a d", p=128)
                q_v = qT.rearrange("(a p) q -> p a q", p=128)
                wk_v = wk.rearrange("(a p) d -> p a d", p=128)
                kv_v = kvT.rearrange("(a p) q -> p a q", p=128)
                for c in range(CP):
                    nc.sync.dma_start(wq_sb[:, c, :], wq_v[:, c, :])
                    nc.sync.dma_start(q_sb[:, c, :], q_v[:, c, :])
                for c in range(CP):
                    nc.sync.dma_start(wk_sb[:, c, :], wk_v[:, c, :])
                    nc.sync.dma_start(kv_sb[:, c, :], kv_v[:, c, :])
                nc.sync.dma_start(wv_sb[:], wv.rearrange("(a p) d -> p a d", p=128))

                def emit_QK(p):
                    """Q and K projection for head-pair p."""
                    for w_sb, x_sb, dst in ((wq_sb, q_sb, QT), (wk_sb, kv_sb, KT)):
                        ps = PS.tile([128, L], f32, tag="sp")
                        for c in range(CP):
                            for hf in range(2):
                                nc.tensor.matmul(
                                    ps[:, hf * 512:(hf + 1) * 512],
                                    w_sb[:, c, p * 128:(p + 1) * 128],
                                    x_sb[:, c, hf * 512:(hf + 1) * 512],
                                    start=(c == 0),
                                    stop=(c == CP - 1),
                                )
                        nc.vector.tensor_copy(dst[:, p, :], ps[:])

                # ---------------- attention + V-proj, software-pipelined ----
                pt_tiles = {}    # (h, k) -> pt tile
                xtu = [None] * H
                rs_pair = {}     # pair p -> [128, 2, 8] rowsum tile

                def emit_S_chunk(h, k):
                    """One S^T k-chunk for head h: matmul, exp, mul by epos."""
                    p, sub = divmod(h, 2)
                    s_ps = PS.tile([128, L], f32, tag="sp")
                    for hf in range(2):
                        nc.tensor.matmul(
                            s_ps[:, hf * 512:(hf + 1) * 512],
                            KT[sub * 64:(sub + 1) * 64, p, k * 128:(k + 1) * 128],
                            QT[sub * 64:(sub + 1) * 64, p, hf * 512:(hf + 1) * 512],
                        )
                    pr = prp.tile([128, L], f16, tag="pr")
                    nc.scalar.activation(pr[:], s_ps[:], AF.Exp, bias=expb[:])
                    ep = eposp.tile([128, L], f16, tag="ep")
                    nc.sync.dma_start(ep[:], epos[h, k * 128:(k + 1) * 128, :])
                    pt = ptp.tile([128, L], f16, name=f"pt{h}_{k}", tag="pt")
                    # offload one multiply per head to the otherwise-idle GpSimd
                    eng = nc.gpsimd if k == 3 else nc.vector
                    eng.tensor_mul(pt[:], pr[:], ep[:])
                    pt_tiles[(h, k)] = pt

                def emit_V(k):
                    ps = PS.tile([128, DIM], f32, tag="op")
                    for c in range(CP):
                        for lo, sz in ((0, 512), (512, 256)):
                            nc.tensor.matmul(
                                ps[:, lo:lo + sz],
                                kv_sb[:, c, k * 128:(k + 1) * 128],
                                wv_sb[:, c, lo:lo + sz],
                                start=(c == 0),
                                stop=(c == CP - 1),
                            )
                    nc.vector.memset(Vt[k][:, :, HD:HD + 1], 1.0)
                    nc.vector.tensor_copy(
                        Vt[k][:, :, 0:HD],
                        ps[:].rearrange("p (h d) -> p h d", d=HD),
                    )

                o_ps_cur = [None]

                def emit_O_chunk(h, k):
                    """One O^T accumulation step (k-chunk) for head h."""
                    if k == 0:
                        o_ps_cur[0] = PS.tile([65, L], f32, name=f"ops{h}", tag="op")
                    o_ps = o_ps_cur[0]
                    pt = pt_tiles.pop((h, k))
                    for hf in range(2):
                        nc.tensor.matmul(
                            o_ps[:, hf * 512:(hf + 1) * 512],
                            Vt[k][:, h, :],
                            pt[:, hf * 512:(hf + 1) * 512],
                            start=(k == 0),
                            stop=(k == KC - 1),
                        )
                    if k == KC - 1:
                        xtu[h] = xtup.tile([65, L], f32, name=f"xtu{h}", tag="xtu")
                        nc.vector.tensor_copy(xtu[h][:], o_ps[:])
                        nc.sync.dma_start(rscr[h:h + 1, :], xtu[h][64:65, :])

                # ---- normalize machinery, staged to avoid queue-head stalls:
                # stage A (iter 2p+2): gather pair rowsums to [128, 2, 8]
                # stage B (iter 2p+2, late): reciprocal + scatter + bc loads
                # stage C (iter 2p+3): normalize muls (GpSimd; bc landed)
                bcs = {}

                def norm_stage_A(p):
                    h0 = 2 * p
                    rs_pair[p] = rsp.tile([128, 2, 8], f32, name=f"rs{p}", tag="rs")
                    nc.sync.dma_start(
                        rs_pair[p][:],
                        rscr[h0:h0 + 2, :].rearrange("h (p a) -> p h a", p=128),
                    )

                def norm_stage_B(p, recip_eng):
                    rc_t = rsp.tile([128, 2, 8], f32, name=f"rc{p}", tag="rc")
                    recip_eng.reciprocal(rc_t[:], rs_pair.pop(p)[:])
                    for sub in range(2):
                        h = 2 * p + sub
                        nc.sync.dma_start(rscr2[h], rc_t[:, sub, :])
                        bc = bcp.tile([64, L], f32, name=f"bc{h}", tag="bc")
                        nc.sync.dma_start(
                            bc[:],
                            rscr2[h]
                            .rearrange("p a -> (p a)")
                            .unsqueeze(0)
                            .broadcast_to([64, L]),
                        )
                        bcs[h] = bc

                def norm_stage_C(p, eng):
                    for sub in range(2):
                        h = 2 * p + sub
                        eng.tensor_mul(
                            XT[sub * 64:(sub + 1) * 64, p, :],
                            xtu[h][0:64, :],
                            bcs.pop(h),
                        )
                        xtu[h] = None

                # ---- head-pair projections spread across iterations as PE
                # filler on the spare "op" PSUM slot: 2 MMs per k-step.
                def make_proj_job(p, which):
                    return {"p": p, "which": which, "ps": None}

                def emit_proj_step(job, c):
                    if job is None or c >= CP:
                        return
                    p = job["p"]
                    w_sb, x_sb, dst = (
                        (wq_sb, q_sb, QT)
                        if job["which"] == "Q"
                        else (wk_sb, kv_sb, KT)
                    )
                    if c == 0:
                        job["ps"] = PS.tile(
                            [128, L], f32, name=f"pj{p}{job['which']}", tag="op"
                        )
                    for hf in range(2):
                        nc.tensor.matmul(
                            job["ps"][:, hf * 512:(hf + 1) * 512],
                            w_sb[:, c, p * 128:(p + 1) * 128],
                            x_sb[:, c, hf * 512:(hf + 1) * 512],
                            start=(c == 0),
                            stop=(c == CP - 1),
                        )
                    if c == CP - 1:
                        nc.vector.tensor_copy(dst[:, p, :], job["ps"][:])

                # ---------------- front ----------------
                emit_QK(0)
                for k0 in range(0, KC, 2):
                    emit_S_chunk(0, k0)
                    emit_S_chunk(0, k0 + 1)
                    emit_V(k0)
                    emit_V(k0 + 1)
                emit_QK(1)
                for k in range(KC):
                    emit_S_chunk(1, k)

                # Q/K projections of pair p: Q at iter 2p-3, K at 2p-2
                proj_sched = {}
                for p in range(2, CP):
                    proj_sched[2 * p - 3] = make_proj_job(p, "Q")
                    proj_sched[2 * p - 2] = make_proj_job(p, "K")

                # ---------------- steady state ----------------
                # out-proj d-chunks 0-2 partials (+bias), computed early on
                # the spare op PSUM slot and parked fp16 in the dead Q/K
                # weight tiles (their last read is the pair-5 projection).
                def partial_ap(qc):
                    return (
                        wq_sb[:, qc, :] if qc < CP else wk_sb[:, qc - CP, :]
                    )

                def emit_outproj_part1(qc):
                    ps = PS.tile([128, DIM], f32, name=f"po{qc}", tag="op")
                    for d in range(3):
                        for lo, sz in ((0, 512), (512, 256)):
                            nc.tensor.matmul(
                                ps[:, lo:lo + sz],
                                XT[:, d, qc * 128:(qc + 1) * 128],
                                wp_sb[:, d, lo:lo + sz],
                                start=(d == 0),
                                stop=(d == 2),
                            )
                    nc.vector.tensor_add(partial_ap(qc), ps[:], bias_bc[:])

                outproj_q = iter(range(KC))
                for h in range(1, H):
                    job = proj_sched.get(h)
                    for k0 in range(0, KC, 2):
                        # batch same-PE-array-mode matmuls (S = 64-row tiled,
                        # O/proj = full 128) to halve mode-switch drains
                        for k in (k0, k0 + 1):
                            if h + 1 < H:
                                emit_S_chunk(h + 1, k)
                        for k in (k0, k0 + 1):
                            emit_O_chunk(h - 1, k)
                            emit_proj_step(job, k)
                        k = k0 + 1
                        # pair p: rowsums complete at end of iter 2p+2 ->
                        # A/B at iter 2p+3 (odd), C at iter 2p+4 (even)
                        if k == 1 and h % 2 == 1 and h >= 3:
                            norm_stage_A((h - 3) // 2)
                        if k == 5 and h % 2 == 1 and h >= 3:
                            norm_stage_B((h - 3) // 2, nc.vector)
                        if k == 5 and h % 2 == 0 and h >= 4:
                            norm_stage_C((h - 4) // 2, nc.gpsimd)
                        # out-proj d0-2 partials as PE filler in iters 9-10
                        # (pairs 0-2 normalized by iter 8; few proj jobs left)
                        if h in (9, 10) and k in (3, 7):
                            emit_outproj_part1(next(outproj_q))
                # last head's O interleaved with the remaining partials
                for k in range(KC):
                    emit_O_chunk(H - 1, k)
                    if k % 2 == 1:
                        emit_outproj_part1(next(outproj_q))
                # tail: finish pairs 4 and 5 immediately on DVE
                norm_stage_C(H // 2 - 2, nc.vector)
                norm_stage_A(H // 2 - 1)
                norm_stage_B(H // 2 - 1, nc.vector)
                norm_stage_C(H // 2 - 1, nc.vector)

                # ---------------- phase 3: output projection (d 3-5) --------
                # alternate PSUM tags so 4 accumulations are in flight
                for qc in range(KC):
                    ps = PS.tile([128, DIM], f32, tag="op" if qc % 2 == 0 else "sp")
                    for d in range(3, CP):
                        for lo, sz in ((0, 512), (512, 256)):
                            nc.tensor.matmul(
                                ps[:, lo:lo + sz],
                                XT[:, d, qc * 128:(qc + 1) * 128],
                                wp_sb[:, d, lo:lo + sz],
                                start=(d == 3),
                                stop=(d == CP - 1),
                            )
                    ot = outp.tile([128, DIM], f32, name=f"oo{qc}", tag="ot")
                    nc.vector.tensor_add(ot[:], ps[:], partial_ap(qc))
                    nc.sync.dma_start(out[qc * 128:(qc + 1) * 128, :], ot[:])

    nc.compile()
    return nc


def _get_nc():
    if "nc" not in _CACHE:
        _CACHE["nc"] = _build()
    return _CACHE["nc"]


def _host_prep(q, kv, attn_pos, Wq, Wkv, Wproj, bproj):
    q = np.asarray(q, dtype=np.float32)
    kv = np.asarray(kv, dtype=np.float32)
    attn_pos = np.asarray(attn_pos, dtype=np.float32)
    Wq = np.asarray(Wq, dtype=np.float32)
    Wkv = np.asarray(Wkv, dtype=np.float32)
    Wproj = np.asarray(Wproj, dtype=np.float32)
    bproj = np.asarray(bproj, dtype=np.float32)

    wq = np.ascontiguousarray((Wq * SCALE).T).astype(np.float16)   # [c, d]
    wk = np.ascontiguousarray(Wkv[:DIM].T).astype(np.float16)      # [c, d]
    wv = np.ascontiguousarray(Wkv[DIM:].T).astype(np.float16)      # [c, d]
    wp = np.ascontiguousarray(Wproj.T).astype(np.float16)          # [d, e]
    bias = np.ascontiguousarray(np.tile(bproj[None, :], (128, 1)))
    # epos[h, k, q] = exp(attn_pos[0, h, q, k])
    epos = np.ascontiguousarray(
        np.exp(attn_pos[0]).transpose(0, 2, 1)
    ).astype(np.float16)

    qT = np.ascontiguousarray(q.transpose(0, 2, 1)).astype(np.float16)   # [B, c, L]
    kvT = np.ascontiguousarray(kv.transpose(0, 2, 1)).astype(np.float16)

    shared = {"wq": wq, "wk": wk, "wv": wv, "wp": wp, "bias": bias, "epos": epos}
    in_maps = []
    for b in range(B):
        m = dict(shared)
        m["qT"] = qT[b]
        m["kvT"] = kvT[b]
        in_maps.append(m)
    return in_maps


def kernel(q, kv, attn_pos, Wq, Wkv, Wproj, bproj):
    from concourse.bass_utils import run_bass_kernel_spmd

    nc = _get_nc()
    in_maps = _host_prep(q, kv, attn_pos, Wq, Wkv, Wproj, bproj)
    res = run_bass_kernel_spmd(nc, in_maps, list(range(NCORES)))
    return np.stack([res.results[b]["out"] for b in range(B)], axis=0)
